# revision 9
# baseline (speedup 1.0000x reference)
"""Trainium2 Bass kernel for nn_CriticEncoder (2-layer LSTM + causal MHA attn-gate).

Strategy: data-parallel over batch across 8 cores (b=4 per core), everything
core-local. Per core:
  P1: gates0_in.T = Wih0r @ x.T  (weight-stationary matmul -> DRAM gin stream)
  P2: SOFTWARE-PIPELINED dual recurrence: one hardware loop runs layer-0 and
      layer-1 LSTM steps interleaved (layer 1 lags by 2 U-blocks). Layer 1's
      input gates gin1 = Wih1 @ h0 are produced incrementally on-chip from
      freshly written h0 blocks (no DRAM round-trip). Each layer's serial
      pointwise tail hides under the other layer's LDWEIGHTS/matmul stream,
      so the loop runs at the weight-load roofline.
      Recurrent + layer-1 input weights are fp8e4 (FWL loads 4 fp8/read vs
      2 bf16) scaled by WSC host-side; ACT scale undoes it.
  P3: fused attention: qT/kT projections, per-(sample,head-pair) scores on PE,
      exp on ACT (scores are tiny -> no max subtraction), causal mask by
      block structure + tril on diagonal blocks, numer/denom reduction fused
      with the (attn_w * h) @ Wo.T contraction (key-time index == hidden index
      since L == H).
State, gates and softmax math are fp32.
"""

import numpy as np
import ml_dtypes
from contextlib import ExitStack

import concourse.bass as bass
import concourse.tile as tile
from concourse import bacc, mybir
from concourse.bass import ds
from concourse.bass_utils import run_bass_kernel_spmd

F32 = mybir.dt.float32
BF16 = mybir.dt.bfloat16
FP8 = mybir.dt.float8e4
AF = mybir.ActivationFunctionType
AX = mybir.AxisListType
BF16NP = ml_dtypes.bfloat16
FP8NP = ml_dtypes.float8_e4m3
WSC = 512.0
IWSC = 1.0 / WSC

E, H, L_FULL, B, NH, HD = 256, 512, 512, 32, 8, 64
G = 4 * H
P = 128
NCORES = 8
BPC = B // NCORES
KCH = H // P   # 4
MCH = G // P   # 16
ECH = E // P   # 2
U = 8          # recurrence steps per block
JB = 4         # blocks per For_i body
LAG = 2        # layer-1 lag in blocks
SH = LAG * U   # hT1 time index shift


def build_program(L=L_FULL, bpc=BPC, n_devices=NCORES, reps=1,
                  do_proj=True, do_rec=True, do_attn=True):
    nc = bacc.Bacc("TRN2", target_bir_lowering=False, debug=False,
                   num_devices=n_devices)
    TCH = L // P
    NB = L // U                      # real blocks per layer
    NPOS = NB + LAG                  # pipeline positions
    NPOS += (-NPOS) % JB             # round to whole bodies
    LP = NPOS * U                    # padded step count
    assert L % U == 0 and L % P == 0

    def din(name, shape, dt):
        return nc.dram_tensor(name, shape, dt, kind="ExternalInput").ap()

    xT = din("xT", [P, ECH, L, bpc], BF16)
    Wih0T = din("Wih0T", [P, ECH, MCH, P], BF16)
    Whh0T = din("Whh0T", [P, KCH, MCH, P], FP8)
    Wih1T = din("Wih1T", [P, KCH, MCH, P], BF16)
    Whh1T = din("Whh1T", [P, KCH, MCH, P], FP8)
    WqT = din("WqT", [P, KCH, KCH, P], BF16)
    WkT = din("WkT", [P, KCH, KCH, P], BF16)
    WoD = din("WoD", [P, KCH, P], BF16)
    b0 = din("b0", [P, MCH], F32)
    b1 = din("b1", [P, MCH], F32)
    bq = din("bq", [P, KCH], F32)
    bk = din("bk", [P, KCH], F32)
    bo = din("bo", [P, 1], F32)
    tril = din("tril", [P, P], F32)
    negm2 = din("negm2", [P, 2, P], F32)
    out = nc.dram_tensor("out", [bpc, L, 1], F32, kind="ExternalOutput").ap()
    g0buf = nc.dram_tensor("g0buf", [MCH, P, LP, bpc], F32).ap()

    with tile.TileContext(nc) as tc, ExitStack() as ctx:
        persist = ctx.enter_context(tc.tile_pool(name="persist", bufs=1))
        wk = ctx.enter_context(tc.tile_pool(name="wk", bufs=3))
        big = ctx.enter_context(tc.tile_pool(name="big", bufs=4))
        pj = ctx.enter_context(tc.tile_pool(name="pj", bufs=2))
        atp = ctx.enter_context(tc.tile_pool(name="atp", bufs=2))
        ps_pool = ctx.enter_context(tc.tile_pool(name="ps", bufs=3, space="PSUM"))
        ps_sc = ctx.enter_context(tc.tile_pool(name="ps_sc", bufs=2, space="PSUM"))

        def load_const(ap_in, shape, dt, tag):
            t = persist.tile(shape, dt, tag=tag)
            nc.sync.dma_start(out=t[:], in_=ap_in)
            return t

        sxT = load_const(xT, [P, ECH, L, bpc], BF16, "sxT")
        sWih0 = load_const(Wih0T, [P, ECH, MCH, P], BF16, "sWih0")
        sWhh0 = load_const(Whh0T, [P, KCH, MCH, P], FP8, "sWhh0")
        sWih1 = load_const(Wih1T, [P, KCH, MCH, P], BF16, "sWih1")
        sWhh1 = load_const(Whh1T, [P, KCH, MCH, P], FP8, "sWhh1")
        sWqT = load_const(WqT, [P, KCH, KCH, P], BF16, "sWqT")
        sWkT = load_const(WkT, [P, KCH, KCH, P], BF16, "sWkT")
        sWoD = load_const(WoD, [P, KCH, P], BF16, "sWoD")
        sb0 = load_const(b0, [P, MCH], F32, "sb0")
        sb1 = load_const(b1, [P, MCH], F32, "sb1")
        sbq = load_const(bq, [P, KCH], F32, "sbq")
        sbk = load_const(bk, [P, KCH], F32, "sbk")
        sbo = load_const(bo, [P, 1], F32, "sbo")
        stril = load_const(tril, [P, P], F32, "stril")
        snegm = load_const(negm2, [P, 2, P], F32, "snegm")

        hT0 = persist.tile([P, KCH, LP, bpc], BF16, tag="hT0")
        hT1 = persist.tile([P, KCH, LP, bpc], BF16, tag="hT1")
        if not do_rec:
            nc.vector.memset(hT0[:], 0.0)
            nc.vector.memset(hT1[:], 0.0)

        # ---------- input projection (layer 0, all timesteps) ----------
        def proj_to_gbuf(Wsb, kch, rhs_fn, bias_sb, gbuf):
            ncols = L * bpc
            CB = min(512, ncols)
            tpb = CB // bpc
            for m in range(MCH):
                for n in range(ncols // CB):
                    ps = ps_pool.tile([P, CB], F32, tag="ps_mm")
                    for k in range(kch):
                        nc.tensor.matmul(ps[:], Wsb[:, k, m, :], rhs_fn(k, n, tpb),
                                         start=(k == 0), stop=(k == kch - 1))
                    sb = pj.tile([P, CB], F32, tag="sb_proj")
                    nc.vector.tensor_scalar_add(sb[:], ps[:],
                                                bias_sb[:, m:m + 1])
                    nc.sync.dma_start(
                        out=gbuf[m, :, n * tpb:(n + 1) * tpb, :],
                        in_=sb[:].rearrange("p (t b) -> p t b", b=bpc))

        # zero the pipeline-pad tail of g0buf once
        if do_rec:
            zt = persist.tile([P, LP - L, bpc], F32, tag="zpad")
            nc.vector.memset(zt[:], 0.0)
            for m in range(MCH):
                nc.sync.dma_start(out=g0buf[m, :, L:LP, :], in_=zt[:])

        for _rep in range(reps):
            if do_proj:
                proj_to_gbuf(
                    sWih0, ECH,
                    lambda k, n, tpb: sxT[:, k, n * tpb:(n + 1) * tpb, :]
                    .rearrange("p t b -> p (t b)"),
                    sb0, g0buf)

            # ---------- interleaved dual recurrence ----------
            if do_rec:
                c0 = persist.tile([P, KCH, bpc], F32, tag="c0")
                h0 = persist.tile([P, KCH, 2, bpc], BF16, tag="hst0")
                c1 = persist.tile([P, KCH, bpc], F32, tag="c1")
                h1 = persist.tile([P, KCH, 2, bpc], BF16, tag="hst1")
                for t in (c0, h0, c1, h1):
                    nc.vector.memset(t[:], 0.0)
                gin0s = [persist.tile([P, MCH, U, bpc], F32, tag=f"gin0_{j}",
                                      name=f"gin0_{j}") for j in range(JB)]
                gin1s = [persist.tile([P, MCH, U, bpc], F32, tag=f"gin1_{j}",
                                      name=f"gin1_{j}") for j in range(JB)]
                stg = [persist.tile([P, KCH, U * bpc], BF16, tag=f"stg{j}",
                                    name=f"stg{j}") for j in range(JB)]
                for t in gin1s:
                    nc.vector.memset(t[:], 0.0)

                def lstm_step(Wsb, gin_ap, c_st, h_st, hT, s_idx, t0):
                    rd_sl = s_idx % 2
                    wr_sl = 1 - rd_sl
                    ps = ps_pool.tile([P, MCH, bpc], F32, tag="ps_mm")
                    for m in range(MCH):
                        for k in range(KCH):
                            nc.tensor.matmul(ps[:, m, :], Wsb[:, k, m, :],
                                             h_st[:, k, rd_sl, :],
                                             start=(k == 0), stop=(k == KCH - 1))
                    gf = wk.tile([P, MCH, bpc], F32, tag="gf")
                    nc.vector.tensor_add(gf[:], ps[:], gin_ap)
                    sg = wk.tile([P, 12, bpc], F32, tag="sg")
                    nc.scalar.activation(sg[:], gf[:, 0:12, :], AF.Sigmoid,
                                         scale=IWSC)
                    tg = wk.tile([P, KCH, bpc], F32, tag="tg")
                    nc.scalar.activation(tg[:], gf[:, 12:16, :], AF.Tanh,
                                         scale=IWSC)
                    t1 = wk.tile([P, KCH, bpc], F32, tag="t1")
                    nc.vector.tensor_mul(t1[:], sg[:, 0:4, :], tg[:])
                    t2 = wk.tile([P, KCH, bpc], F32, tag="t2")
                    nc.vector.tensor_mul(t2[:], sg[:, 4:8, :], c_st[:])
                    nc.vector.tensor_add(c_st[:], t1[:], t2[:])
                    tch = wk.tile([P, KCH, bpc], F32, tag="tch")
                    nc.scalar.activation(tch[:], c_st[:], AF.Tanh)
                    nc.vector.tensor_mul(h_st[:, :, wr_sl, :],
                                         sg[:, 8:12, :], tch[:])
                    nc.gpsimd.tensor_copy(
                        hT[:, :, ds(t0 + s_idx, 1), :]
                        .rearrange("p k o b -> p k (o b)"),
                        h_st[:, :, wr_sl, :])

                _kw = {'staggered_reset': True,
                       'hint_engines': (mybir.EngineType.PE,)}
                with tc.For_i(0, LP, JB * U, **_kw) as t0:
                    for j in range(JB):
                        nc.sync.dma_start(
                            out=gin0s[j][:],
                            in_=g0buf[:, :, ds(t0 + j * U, U), :]
                            .rearrange("m p t b -> p m t b"))
                    for j in range(JB):
                        for u in range(U):
                            s_idx = j * U + u
                            lstm_step(sWhh0, gin0s[j][:, :, u, :], c0, h0,
                                      hT0, s_idx, t0)
                            lstm_step(sWhh1, gin1s[(j + LAG) % JB][:, :, u, :],
                                      c1, h1, hT1, s_idx, t0)
                        # gin1 for layer-1 time block (t0+j): Wih1 @ h0-block
                        nc.sync.dma_start(
                            out=stg[j][:],
                            in_=hT0[:, :, ds(t0 + j * U, U), :]
                            .rearrange("p k t b -> p k (t b)"))
                        psj = ps_pool.tile([P, MCH, U * bpc], F32, tag="ps_mm")
                        for m in range(MCH):
                            for k in range(KCH):
                                nc.tensor.matmul(psj[:, m, :], sWih1[:, k, m, :],
                                                 stg[j][:, k, :],
                                                 start=(k == 0),
                                                 stop=(k == KCH - 1))
                        for m in range(MCH):
                            nc.vector.tensor_scalar_add(
                                gin1s[j][:, m, :, :]
                                .rearrange("p t b -> p (t b)"),
                                psj[:, m, :], sb1[:, m:m + 1])

            # ---------- attention + output ----------
            for s in range(bpc if do_attn else 0):
                qT = atp.tile([P, KCH, L], BF16, tag="qTs")
                kT = atp.tile([P, KCH, L], BF16, tag="kTs")
                for (Wp, bvec, dst, tg_) in ((sWqT, sbq, qT, "ps_mm"),
                                             (sWkT, sbk, kT, "ps_mm")):
                    for m in range(KCH):
                        psq = ps_pool.tile([P, L], F32, tag=tg_)
                        for k in range(KCH):
                            nc.tensor.matmul(psq[:], Wp[:, k, m, :],
                                             hT1[:, k, SH:SH + L, s],
                                             start=(k == 0), stop=(k == KCH - 1))
                        nc.vector.tensor_scalar_add(dst[:, m, :], psq[:],
                                                    bvec[:, m:m + 1])

                # hw[t_part, tch, hid] = (h1.T)^T scaled by Wo  (per sample)
                hw = atp.tile([P, TCH, H], F32, tag="hw")
                for r in range(KCH):
                    for c in range(TCH):
                        pst = ps_pool.tile([P, P], F32, tag="ps_mm")
                        nc.tensor.matmul(pst[:],
                                         hT1[:, r, SH + c * P:SH + (c + 1) * P, s],
                                         sWoD[:, r, :], start=True, stop=True)
                        nc.vector.tensor_copy(hw[:, c, r * P:(r + 1) * P], pst[:])

                for qt in range(TCH):
                    ncols = (qt + 1) * P
                    nacc = wk.tile([P, NH], F32, tag="nacc")
                    dacc = wk.tile([P, NH], F32, tag="dacc")
                    for hp in range(NH // 2):
                        pss = ps_sc.tile([P, 2, 512], F32, tag="ps_s")
                        for hh in range(2):
                            nc.tensor.matmul(
                                pss[:, hh, 0:ncols],
                                qT[hh * 64:(hh + 1) * 64, hp, qt * P:(qt + 1) * P],
                                kT[hh * 64:(hh + 1) * 64, hp, 0:ncols],
                                start=True, stop=True)
                        # causal mask: add -1e4 above the diagonal (in PSUM)
                        nc.vector.scalar_tensor_tensor(
                            pss[:, :, qt * P:ncols], pss[:, :, qt * P:ncols],
                            1.0, snegm[:],
                            op0=mybir.AluOpType.mult, op1=mybir.AluOpType.add)
                        for hh in range(2):
                            h_idx = 2 * hp + hh
                            Ee = big.tile([P, 512], F32, tag="Ee")
                            nc.scalar.activation(Ee[:, 0:ncols],
                                                 pss[:, hh, 0:ncols],
                                                 AF.Exp, scale=0.125,
                                                 accum_out=dacc[:, h_idx:h_idx + 1])
                            Em = big.tile([P, 512], F32, tag="Em")
                            nc.vector.tensor_mul(Em[:, 0:ncols], Ee[:, 0:ncols],
                                                 hw[:, qt, 0:ncols])
                            nc.vector.reduce_sum(nacc[:, h_idx:h_idx + 1],
                                                 Em[:, 0:ncols], axis=AX.X)
                    rd = wk.tile([P, NH], F32, tag="rdt")
                    nc.vector.reciprocal(rd[:], dacc[:])
                    pr = wk.tile([P, NH], F32, tag="pr")
                    nc.vector.tensor_mul(pr[:], nacc[:], rd[:])
                    osum = wk.tile([P, 1], F32, tag="osum")
                    nc.vector.reduce_sum(osum[:], pr[:], axis=AX.X)
                    oo = wk.tile([P, 1], F32, tag="oo")
                    nc.vector.tensor_scalar(oo[:], osum[:], 0.125, sbo[:, 0:1],
                                            op0=mybir.AluOpType.mult,
                                            op1=mybir.AluOpType.add)
                    nc.sync.dma_start(out=out[s, qt * P:(qt + 1) * P, :], in_=oo[:])

    nc.compile()
    return nc


def _reorder_rows(W):
    # gate order i,f,g,o -> i,f,o,g so sigmoid block is contiguous
    return np.concatenate([W[0:H], W[H:2 * H], W[3 * H:4 * H], W[2 * H:3 * H]], 0)


def _wT_layout(Wr, kch):
    # [G, K] -> lhsT tiles [P, kch, MCH, P]
    return np.ascontiguousarray(
        Wr.T.reshape(kch, P, MCH, P).transpose(1, 0, 2, 3))


def prep_shared_inputs(inputs, L=L_FULL):
    f = {}
    f["Wih0T"] = _wT_layout(_reorder_rows(inputs["Wih0"] * WSC), ECH).astype(BF16NP)
    f["Whh0T"] = _wT_layout(_reorder_rows(inputs["Whh0"] * WSC), KCH).astype(FP8NP)
    f["Wih1T"] = _wT_layout(_reorder_rows(inputs["Wih1"] * WSC), KCH).astype(BF16NP)
    f["Whh1T"] = _wT_layout(_reorder_rows(inputs["Whh1"] * WSC), KCH).astype(FP8NP)
    f["WqT"] = np.ascontiguousarray(
        inputs["Wq"].T.reshape(KCH, P, KCH, P).transpose(1, 0, 2, 3)).astype(BF16NP)
    f["WkT"] = np.ascontiguousarray(
        inputs["Wk"].T.reshape(KCH, P, KCH, P).transpose(1, 0, 2, 3)).astype(BF16NP)
    wod = np.zeros((P, KCH, P), np.float32)
    for r in range(KCH):
        wod[:, r, :] = np.diag(inputs["Wo"][0, r * P:(r + 1) * P])
    f["WoD"] = wod.astype(BF16NP)
    b0r = _reorder_rows(((inputs["bih0"] + inputs["bhh0"]) * WSC).reshape(4 * H, 1))[:, 0]
    b1r = _reorder_rows(((inputs["bih1"] + inputs["bhh1"]) * WSC).reshape(4 * H, 1))[:, 0]
    f["b0"] = np.ascontiguousarray(b0r.reshape(MCH, P).T).astype(np.float32)
    f["b1"] = np.ascontiguousarray(b1r.reshape(MCH, P).T).astype(np.float32)
    f["bq"] = np.ascontiguousarray(
        inputs["bq"].reshape(KCH, P).T).astype(np.float32)
    f["bk"] = np.ascontiguousarray(
        inputs["bk"].reshape(KCH, P).T).astype(np.float32)
    f["bo"] = np.full((P, 1), np.float32(inputs["bo"][0]), np.float32)
    f["tril"] = np.tril(np.ones((P, P), np.float32))
    f["negm2"] = np.broadcast_to(
        (-1e4 * (1.0 - f["tril"]))[:, None, :], (P, 2, P)).copy()
    return f


def prep_xT(x_slice, L, bpc):
    # [bpc, L, E] -> [P, ECH, L, bpc]
    return np.ascontiguousarray(
        x_slice.transpose(2, 1, 0).reshape(ECH, P, L, bpc)
        .transpose(1, 0, 2, 3)).astype(BF16NP)


_CACHE = {}


def kernel(**inputs):
    inputs = {k: np.asarray(v) for k, v in inputs.items()}
    if "nc" not in _CACHE:
        _CACHE["nc"] = build_program()
    nc = _CACHE["nc"]
    shared = prep_shared_inputs(inputs)
    in_maps = []
    for c in range(NCORES):
        m = dict(shared)
        m["xT"] = prep_xT(inputs["x"][c * BPC:(c + 1) * BPC], L_FULL, BPC)
        in_maps.append(m)
    res = run_bass_kernel_spmd(nc, in_maps, core_ids=list(range(NCORES)))
    out = np.concatenate([res.results[c]["out"] for c in range(NCORES)], 0)
    return out.astype(np.float32)


if __name__ == "__main__":
    # smoke: random inputs with the right shapes
    rng = np.random.default_rng(0)
    s = np.float32(0.02)
    inp = dict(
        x=rng.standard_normal((B, L_FULL, E)).astype(np.float32),
        Wih0=(rng.standard_normal((G, E)).astype(np.float32) * s),
        Whh0=(rng.standard_normal((G, H)).astype(np.float32) * s),
        bih0=np.zeros(G, np.float32), bhh0=np.zeros(G, np.float32),
        Wih1=(rng.standard_normal((G, H)).astype(np.float32) * s),
        Whh1=(rng.standard_normal((G, H)).astype(np.float32) * s),
        bih1=np.zeros(G, np.float32), bhh1=np.zeros(G, np.float32),
        Wq=(rng.standard_normal((H, H)).astype(np.float32) * s),
        bq=np.zeros(H, np.float32),
        Wk=(rng.standard_normal((H, H)).astype(np.float32) * s),
        bk=np.zeros(H, np.float32),
        Wo=(rng.standard_normal((1, H)).astype(np.float32) * s),
        bo=np.zeros(1, np.float32),
    )
    got = kernel(**inp)
    print("kernel out shape:", got.shape, got.dtype)
